# revision 1
# baseline (speedup 1.0000x reference)
"""BCMSE loss kernel for 8 Trainium2 NeuronCores.

Strategy (pure data parallel, memory-bound):
  - Shard the [B, 9] inputs along batch across 8 cores (B/8 rows each).
  - Host-side, each shard is transposed to column-major [9, S] with rows
    permuted to [0,3,6,7,8,1,2,4,5] so that on device every column group
    (scalar {0,3}, vec {6,7,8}, angle {1,2,4,5}) is a contiguous block.
  - Each core streams its shard through SBUF in tiles of 128*q rows and
    reduces everything to 5 per-partition partial sums:
      p0   = sum (o-t)^2 over scalar cols
      p1   = sum wrapped-angle err^2 over angle cols
      p2   = sum (vmod-t)^2 over vec cols
      ext  = sum |floor(o)| over angle cols
      nrm  = sum norm over rows
  - Host combines 8x128 partials in float64 and applies the final formula.

Math notes:
  floor(x) = rne(x - 0.5) computed as (x + (1.5*2^23 - 0.5)) - 1.5*2^23 in
  one fused tensor_scalar op (exact except x exactly integral, measure-zero
  for randn inputs).
  angle |err| = min(|d|, ||d|-1|) with d = mod(o,1) - t  (algebraically equal
  to the reference's shortest-path target shift, incl. the |d|=0.5 boundary).
  vec remainder(v, norm) = v + norm*[v<0] exactly, because |v| <= norm.
"""
import numpy as np

import concourse.bacc as bacc
import concourse.mybir as mybir
from concourse.tile import TileContext
from concourse.bass_utils import run_bass_kernel_spmd

N_CORES = 8
BATCH = 4194304
SHARD = BATCH // N_CORES          # 524288 rows per core
P = 128
Q = 512                           # rows per partition per tile
TILE_ROWS = P * Q                 # 65536 rows per tile
N_TILES = SHARD // TILE_ROWS      # 8
PERM = [0, 3, 6, 7, 8, 1, 2, 4, 5]  # scalar(2) | vec(3) | angle(4)
MAGIC = float(1.5 * 2**23)        # rne magic for fp32
MAGIC_H = float(1.5 * 2**10)      # rne magic for fp16
HALF = True                       # ship fp16 to the device
CONSTANT_WEIGHT = 10.0

_cache = {}


def _build(shard, q, n_tiles, reps=1, mode='full', half=False):
    dt = mybir.dt.float16 if half else mybir.dt.float32
    magic = MAGIC_H if half else MAGIC
    f32 = mybir.dt.float32
    nc = bacc.Bacc("TRN2", target_bir_lowering=False)
    # host pre-tiles the data: row i*P+p holds tile i / partition p, 9q floats
    o_d = nc.dram_tensor("o", [n_tiles * P, 9 * q], dt, kind="ExternalInput")
    t_d = nc.dram_tensor("t", [n_tiles * P, 9 * q], dt, kind="ExternalInput")
    out_d = nc.dram_tensor("partials", [P, 8], f32, kind="ExternalOutput")

    with TileContext(nc) as tc:
        with (
            tc.tile_pool(name="io", bufs=3) as io,
            tc.tile_pool(name="scr", bufs=6) as scr,
            tc.tile_pool(name="acc", bufs=1) as acc,
        ):
            neg1 = acc.tile([P, 1], dt, tag="neg1")
            nc.vector.memset(neg1[:], -1.0)
            negM = acc.tile([P, 1], dt, tag="negM")
            nc.vector.memset(negM[:], -magic)
            s_p0 = acc.tile([P, n_tiles], f32, tag="s_p0")
            s_p1 = acc.tile([P, n_tiles], f32, tag="s_p1")
            s_p2 = acc.tile([P, n_tiles], f32, tag="s_p2")
            s_ext = acc.tile([P, n_tiles], f32, tag="s_ext")
            s_nrm = acc.tile([P, n_tiles], f32, tag="s_nrm")
            if mode == 'dma':
                for s in (s_p0, s_p1, s_p2, s_ext, s_nrm):
                    nc.vector.memset(s[:], 0.0)

            from contextlib import nullcontext
            loop = tc.For_i(0, reps, 1) if reps > 1 else nullcontext()
            with loop:
              for i in range(n_tiles):
                ot = io.tile([P, 9 * q], dt, tag="ot")
                tt = io.tile([P, 9 * q], dt, tag="tt")
                wid = 9 * q // 8 if mode == 'nodma' else 9 * q
                nc.sync.dma_start(out=ot[:, 0:wid], in_=o_d[i * P:(i + 1) * P, 0:wid])
                nc.sync.dma_start(out=tt[:, 0:wid], in_=t_d[i * P:(i + 1) * P, 0:wid])
                if mode == 'dma':
                    continue
                # contiguous column-group views (PERM order in DRAM)
                o_sc, t_sc = ot[:, 0:2 * q], tt[:, 0:2 * q]
                o_v, t_v = ot[:, 2 * q:5 * q], tt[:, 2 * q:5 * q]
                o_a, t_a = ot[:, 5 * q:9 * q], tt[:, 5 * q:9 * q]

                # ---- scalar cols: p0 += sum (o-t)^2
                pd = scr.tile([P, 2 * q], dt, tag="pd")
                nc.vector.tensor_sub(out=pd[:], in0=o_sc, in1=t_sc)
                nc.scalar.activation(out=pd[:], in_=pd[:],
                                     func=mybir.ActivationFunctionType.Square,
                                     accum_out=s_p0[:, i:i + 1])

                # ---- angle cols (all-DVE chain; ACT only for the two accums)
                # y = (o - 0.5) + magic; the fp16/fp32 output cast rounds at
                # ulp 1 in the magic range => y = floor(o) + magic
                y = scr.tile([P, 4 * q], dt, tag="y")
                nc.vector.tensor_scalar(out=y[:], in0=o_a,
                                        scalar1=0.5, scalar2=magic,
                                        op0=mybir.AluOpType.subtract,
                                        op1=mybir.AluOpType.add)
                fl = scr.tile([P, 4 * q], dt, tag="fl")
                nc.vector.tensor_scalar(out=fl[:], in0=y[:],
                                        scalar1=magic, scalar2=None,
                                        op0=mybir.AluOpType.subtract)
                nc.scalar.activation(out=y[:], in_=fl[:],
                                     func=mybir.ActivationFunctionType.Abs,
                                     accum_out=s_ext[:, i:i + 1])
                m = scr.tile([P, 4 * q], dt, tag="m")
                nc.vector.tensor_sub(out=m[:], in0=o_a, in1=fl[:])
                d = scr.tile([P, 4 * q], dt, tag="d")
                nc.vector.tensor_sub(out=d[:], in0=m[:], in1=t_a)
                # err = d - clamp(rne(d), -1, 1);  rne via fp32-stage magic
                nc.vector.tensor_scalar(out=m[:], in0=d[:],
                                        scalar1=MAGIC, scalar2=MAGIC,
                                        op0=mybir.AluOpType.add,
                                        op1=mybir.AluOpType.subtract)
                nc.vector.tensor_scalar(out=m[:], in0=m[:],
                                        scalar1=1.0, scalar2=-1.0,
                                        op0=mybir.AluOpType.min,
                                        op1=mybir.AluOpType.max)
                nc.vector.tensor_sub(out=d[:], in0=d[:], in1=m[:])
                nc.scalar.activation(out=d[:], in_=d[:],
                                     func=mybir.ActivationFunctionType.Square,
                                     accum_out=s_p1[:, i:i + 1])

                # ---- vec cols
                sq = scr.tile([P, 3 * q], dt, tag="sq")
                nc.scalar.activation(out=sq[:], in_=o_v,
                                     func=mybir.ActivationFunctionType.Square)
                nc.vector.tensor_add(out=sq[:, 0:q], in0=sq[:, 0:q], in1=sq[:, q:2 * q])
                nc.vector.tensor_add(out=sq[:, 0:q], in0=sq[:, 0:q], in1=sq[:, 2 * q:3 * q])
                nc.scalar.activation(out=sq[:, q:2 * q], in_=sq[:, 0:q],
                                     func=mybir.ActivationFunctionType.Sqrt,
                                     accum_out=s_nrm[:, i:i + 1])
                nrm = sq[:, q:2 * q]
                w = scr.tile([P, 3 * q], dt, tag="w")
                nc.vector.tensor_scalar(out=w[:], in0=o_v, scalar1=0.0,
                                        scalar2=None, op0=mybir.AluOpType.is_lt)
                for c in range(3):
                    nc.vector.tensor_mul(
                        out=w[:, c * q:(c + 1) * q], in0=w[:, c * q:(c + 1) * q],
                        in1=nrm)
                nc.vector.tensor_add(out=w[:], in0=o_v, in1=w[:])
                nc.vector.tensor_sub(out=w[:], in0=w[:], in1=t_v)
                nc.scalar.activation(out=w[:], in_=w[:],
                                     func=mybir.ActivationFunctionType.Square,
                                     accum_out=s_p2[:, i:i + 1])

            out_sb = acc.tile([P, 8], f32, tag="out_sb")
            nc.vector.memset(out_sb[:], 0.0)
            for j, s in enumerate([s_p0, s_p1, s_p2, s_ext, s_nrm]):
                nc.vector.tensor_reduce(out=out_sb[:, j:j + 1], in_=s[:],
                                        axis=mybir.AxisListType.X,
                                        op=mybir.AluOpType.add)
            nc.sync.dma_start(out=out_d[:], in_=out_sb[:])

    nc.compile()
    return nc


def _prep(arr, shard, core, q=Q, half=False):
    # [B, 9] row-major -> per-core pre-tiled [n_tiles*P, 9*q]:
    # row i*P+p = tile i / partition p, holding 9 blocks (PERM col order)
    # of q consecutive batch elements each
    sl = arr[core * shard:(core + 1) * shard, :]
    n_tiles = shard // (P * q)
    a = sl.reshape(n_tiles, P, q, 9).transpose(0, 1, 3, 2)[:, :, PERM, :]
    out = np.ascontiguousarray(a, dtype=np.float16 if half else np.float32)
    return out.reshape(n_tiles * P, 9 * q)


def _finish(partials, batch):
    # partials: [n_cores, 128, 8] fp32 -> final scalar, float64 combine
    tot = partials.astype(np.float64).sum(axis=(0, 1))
    p0, p1, p2, ext, nrm = tot[0], tot[1], tot[2], tot[3], tot[4]
    c0 = ext / batch / CONSTANT_WEIGHT
    c1 = nrm / batch / CONSTANT_WEIGHT
    mse = (p0 + p1 + p2) / (batch * 9)
    if (p0 > p1) and (p0 > p2):
        amount = 0.0
    elif (p0 > p1) and (p0 < p2):
        amount = c1
    elif (p0 < p1) and (p0 > p2):
        amount = c0
    else:
        amount = c0 + c1
    return np.float32(mse + amount)


def _run(outputs, targets, shard, q, n_tiles, n_cores, half=HALF, **spmd_kwargs):
    key = (shard, q, n_tiles, half)
    if key not in _cache:
        _cache[key] = _build(shard, q, n_tiles, half=half)
    nc = _cache[key]
    in_maps = [{"o": _prep(outputs, shard, k, q, half),
                "t": _prep(targets, shard, k, q, half)}
               for k in range(n_cores)]
    br = run_bass_kernel_spmd(nc, in_maps, list(range(n_cores)), **spmd_kwargs)
    partials = np.stack([r["partials"] for r in br.results])
    if spmd_kwargs:
        return partials, br
    return partials


def kernel(outputs, targets):
    outputs = np.asarray(outputs)
    targets = np.asarray(targets)
    assert outputs.shape == (BATCH, 9), outputs.shape
    partials = _run(outputs, targets, SHARD, Q, N_TILES, N_CORES)
    return _finish(partials, BATCH)



# revision 2
# speedup vs baseline: 1.0086x; 1.0086x over previous
"""BCMSE loss kernel for 8 Trainium2 NeuronCores — fused custom-DVE ops.

Design (per core, per tile of [P=128, 9*q] fp16, PERM col order [sc|vec|angle],
angle columns host-biased by -0.5 so floor(o) = rne(o')):

  DVE:
    u_sc   = o_sc - t_sc                    TT @2x        570 cyc
    p1    += BC_WRAPSQ(o'_a, t'_a)          custom 1x    2106   (fused angle)
    y      = o'_a + 1536 (fp16 rne)         TS @4x        570
    s2     = vx^2 + vy^2                    custom 1x     570
    nsq    = s2 + vz^2                      custom 1x     570
    vmod_c = v_c + n*[v_c<0]  (x3)          custom 1x    1710
    w      = vmod - t_v                     TT @2x        826
  ACT:
    p0    += Square(u_sc) accum            1376
    ext   += Abs(y - 1536) accum           2400
    nrm   += Sqrt(nsq) accum -> n           864
    p2    += Square(w) accum               1888

BC_WRAPSQ math: fl = rne(o'); u = o' - t' (= o - t); a = |u - fl| = |m - t|;
e = min(a, |a - 1|); accum e^2.  Identical to the reference's shortest-path
wrap (single +-1 shift, strict > 0.5 threshold, incl. boundary).
"""
import numpy as np

import concourse.bacc as bacc
import concourse.mybir as mybir
from concourse.tile import TileContext
from concourse.bass_utils import run_bass_kernel_spmd

# ---------------- custom DVE op registration (idempotent) ----------------
import concourse.dve_ops as dve_ops
from concourse.dve_ops import DveOp, OPS, CUSTOM_DVE_SPECS, _SUB_OPCODE_FOR_NAME, \
    _CUSTOM_DVE_ROW_BASE, has_src1
from concourse.dve_spec import (
    Spec, Src0, Src1, C0, Zero, AluOp, Bin, lower, minn, sq,
)
from concourse.dve_spec import One
from concourse.dve_uop import DveOpSpec
from operator import add as _add

M32 = float(1.5 * 2**23)


def _mk_op(name, spec, subdim=False):
    """Create + register a DveOp at runtime: assign the next opcode row and
    pin uops_sha from our own lower() output (self-consistent)."""
    if name in _SUB_OPCODE_FOR_NAME:
        return next(o for o in OPS if o.name == name)
    row = _CUSTOM_DVE_ROW_BASE + len(OPS)
    assert row < 0x20, "custom DVE opcode rows exhausted"
    sha = {}
    for ver in ("v3", "v4"):
        try:
            s = DveOpSpec(name=name, opcode=row, uops=lower(spec, ver=ver),
                          rd1_en=has_src1(spec))
            sha[ver] = s.sha(ver)
        except Exception:
            pass
    op = DveOp(name, spec, subdim, uops_sha=sha)
    OPS.append(op)
    _SUB_OPCODE_FOR_NAME[name] = row
    CUSTOM_DVE_SPECS[name] = spec
    return op


def _ref_wrapsq(in0, in1, s0, s1, imm2):
    x = in0.astype(np.float32)
    t = in1.astype(np.float32)
    fl = np.rint(x)
    a = np.abs((x - t) - fl)
    e = np.minimum(a, np.abs(a - 1.0))
    out = (e * e).astype(np.float32)
    return out, out.reshape(out.shape[0], -1).sum(axis=-1, keepdims=True)


def _ref_extabs(in0, in1, s0, s1, imm2):
    fl = np.rint(in0.astype(np.float32))
    out = np.abs(fl)
    return out, out.reshape(out.shape[0], -1).sum(axis=-1, keepdims=True)


def _ref_subsq(in0, in1, s0, s1, imm2):
    d = in0.astype(np.float32) - in1.astype(np.float32)
    out = d * d
    return out, out.reshape(out.shape[0], -1).sum(axis=-1, keepdims=True)


def _ref_sq2(in0, in1, s0, s1, imm2):
    a = in0.astype(np.float32); b = in1.astype(np.float32)
    return a * a + b * b


def _ref_sqa(in0, in1, s0, s1, imm2):
    a = in0.astype(np.float32); b = in1.astype(np.float32)
    return a + b * b


def _ref_vmod(in0, in1, s0, s1, imm2):
    v = in0.astype(np.float32); n = in1.astype(np.float32)
    n = n.reshape(v.shape)
    return v + (v < 0) * n


# fl = (o' + M) - M = rne(o'); u = o' - t'; a = |u - fl|; e = min(a, |a-1|)
_y = Src0 + C0
_fl = _y - C0
_u = Src0 - Src1
_a = Bin(AluOp.ABSOLUTE_DIFF, _u, _fl)
_e = minn(_a, Bin(AluOp.ABSOLUTE_DIFF, _a, One))
BC_WRAPSQ = _mk_op("BC_WRAPSQ", Spec(body=sq(_e), accum=_add,
                                     reference=_ref_wrapsq))
BC_EXTABS = _mk_op("BC_EXTABS", Spec(body=Bin(AluOp.ABSOLUTE_DIFF, _fl, Zero),
                                     accum=_add, reference=_ref_extabs))
BC_SUBSQ = _mk_op("BC_SUBSQ", Spec(body=sq(Src0 - Src1), accum=_add,
                                   reference=_ref_subsq))
BC_SQ2 = _mk_op("BC_SQ2", Spec(body=sq(Src0) + sq(Src1), reference=_ref_sq2))
BC_SQA = _mk_op("BC_SQA", Spec(body=Src0 + sq(Src1), reference=_ref_sqa))
BC_VMOD = _mk_op("BC_VMOD",
                 Spec(body=Src0 + Bin(AluOp.IS_LT, Src0, Zero) * Src1,
                      reference=_ref_vmod))

# ---------------- kernel ----------------
N_CORES = 8
BATCH = 4194304
SHARD = BATCH // N_CORES          # 524288 rows per core
P = 128
Q = 512                           # rows per partition per tile
TILE_ROWS = P * Q
N_TILES = SHARD // TILE_ROWS      # 8
PERM = [0, 3, 6, 7, 8, 1, 2, 4, 5]  # scalar(2) | vec(3) | angle(4)
HALF = True
CONSTANT_WEIGHT = 10.0

_cache = {}


def _qs(shard, q=None):
    """Graduated tile widths (rows-per-partition): small head tiles so
    compute starts early, small tail so the drain is short."""
    total = shard // P
    if q:  # uniform override
        assert total % q == 0
        return [q] * (total // q)
    if total <= 1024:  # small shards (tests): plain <=512 chunks
        out = [512] * (total // 512)
        if total % 512:
            out.append(total % 512)
        return out
    head = [128, 256]
    tail = [256]
    mid_total = total - sum(head) - sum(tail)
    mids = [512] * (mid_total // 512)
    rem = mid_total - 512 * len(mids)
    if rem:
        mids = [rem] + mids
    return head + mids + tail


def _build(shard, q, n_tiles, reps=1, mode='full', half=True):
    dt = mybir.dt.float16 if half else mybir.dt.float32
    f32 = mybir.dt.float32
    AF = mybir.ActivationFunctionType
    qs = _qs(shard)
    qmax = max(qs)
    # DRAM holds each tile's [P, 9*q_i] block contiguous (dest byte order);
    # addressed as [n_units*P, 9*128] rows so every tile is a contiguous
    # row-range memcpy (fully dense DMA, no strided source rows).
    U = 128
    n_units = sum(qs) // U
    nc = bacc.Bacc("TRN2", target_bir_lowering=False)
    o_d = nc.dram_tensor("o", [n_units * P, 9 * U], dt, kind="ExternalInput")
    t_d = nc.dram_tensor("t", [n_units * P, 9 * U], dt, kind="ExternalInput")
    out_d = nc.dram_tensor("partials", [P, 8], f32, kind="ExternalOutput")
    n_tiles = len(qs)

    with TileContext(nc) as tc:
        with (
            tc.tile_pool(name="io", bufs=3) as io,
            tc.tile_pool(name="scr", bufs=3) as scr,
            tc.tile_pool(name="acc", bufs=1) as acc,
        ):
            bneg = acc.tile([P, 1], f32, tag="bneg")
            nc.vector.memset(bneg[:], -1536.0)
            s_p0 = acc.tile([P, n_tiles], f32, tag="s_p0")
            s_p1 = acc.tile([P, n_tiles], f32, tag="s_p1")
            s_p2 = acc.tile([P, n_tiles], f32, tag="s_p2")
            s_ext = acc.tile([P, n_tiles], f32, tag="s_ext")
            s_nrm = acc.tile([P, n_tiles], f32, tag="s_nrm")
            if mode == 'dma':
                for s in (s_p0, s_p1, s_p2, s_ext, s_nrm):
                    nc.vector.memset(s[:], 0.0)

            from contextlib import nullcontext
            loop = tc.For_i(0, reps, 1) if reps > 1 else nullcontext()
            with loop:
                rows = [P * (sum(qs[:k]) // U) for k in range(len(qs))]
                ios = {}

                def load(k):
                    q = qs[k]
                    ot = io.tile([P, 9 * qmax], dt, tag="ot", name="ot")
                    tt = io.tile([P, 9 * qmax], dt, tag="tt", name="tt")
                    nu = P * (q // U) // (8 if mode == 'nodma' else 1)
                    nc.sync.dma_start(out=ot[:, 0:9 * q * nu // (P * (q // U))] if mode == 'nodma' else ot[:, 0:9 * q],
                                      in_=o_d[rows[k]:rows[k] + nu, :])
                    nc.sync.dma_start(out=tt[:, 0:9 * q * nu // (P * (q // U))] if mode == 'nodma' else tt[:, 0:9 * q],
                                      in_=t_d[rows[k]:rows[k] + nu, :])
                    ios[k] = (ot, tt)

                def norm_chain(k):
                    # nsq -> ACT sqrt for tile k (issued one tile early so the
                    # sqrt clears ACT's queue before vmod(k) needs n(k))
                    q = qs[k]
                    ot, _ = ios[k]
                    o_v = ot[:, 2 * q:5 * q]
                    s2 = scr.tile([P, qmax], dt, tag="s2", name="s2")[:, 0:q]
                    nc.vector._custom_dve(BC_SQ2, out=s2, in0=o_v[:, 0:q],
                                          in1=o_v[:, q:2 * q])
                    nsq = scr.tile([P, qmax], dt, tag="nsq", name="nsq")[:, 0:q]
                    nc.vector._custom_dve(BC_SQA, out=nsq, in0=s2,
                                          in1=o_v[:, 2 * q:3 * q])
                    n = scr.tile([P, qmax], dt, tag="n", name="n")[:, 0:q]
                    nc.scalar.activation(out=n, in_=nsq, func=AF.Sqrt,
                                         accum_out=s_nrm[:, k:k + 1])
                    return n

                if mode == 'dma':
                    for k in range(len(qs)):
                        load(k)
                else:
                  load(0)
                  n_cur = norm_chain(0)
                  for i, q in enumerate(qs):
                    if i + 1 < len(qs):
                        load(i + 1)
                    ot, tt = ios.pop(i)
                    o_sc, t_sc = ot[:, 0:2 * q], tt[:, 0:2 * q]
                    o_v, t_v = ot[:, 2 * q:5 * q], tt[:, 2 * q:5 * q]
                    o_a, t_a = ot[:, 5 * q:9 * q], tt[:, 5 * q:9 * q]

                    # angle + scalar groups on DVE while sqrt(i) runs on ACT
                    j4 = scr.tile([P, 4 * qmax], dt, tag="j4", name="j4")[:, 0:4 * q]
                    nc.vector._custom_dve(BC_WRAPSQ, out=j4, in0=o_a, in1=t_a,
                                          s0=M32, accum_out=s_p1[:, i:i + 1])
                    y = scr.tile([P, 4 * qmax], dt, tag="y", name="y")[:, 0:4 * q]
                    nc.vector.tensor_scalar(out=y, in0=o_a, scalar1=1536.0,
                                            scalar2=None, op0=mybir.AluOpType.add)
                    u_sc = scr.tile([P, 2 * qmax], dt, tag="u_sc", name="u_sc")[:, 0:2 * q]
                    nc.vector.tensor_sub(out=u_sc, in0=o_sc, in1=t_sc)

                    # vmod / w with this tile's n; next tile's norm chain goes
                    # ahead of this tile's ACT squares
                    n = n_cur
                    vmod = scr.tile([P, 3 * qmax], dt, tag="vmod", name="vmod")[:, 0:3 * q]
                    n3 = n.unsqueeze(1).to_broadcast((P, 3, q))
                    nc.vector._custom_dve(BC_VMOD, out=vmod, in0=o_v, in1=n3)
                    w = scr.tile([P, 3 * qmax], dt, tag="w", name="w")[:, 0:3 * q]
                    nc.vector.tensor_sub(out=w, in0=vmod, in1=t_v)
                    if i + 1 < len(qs):
                        n_cur = norm_chain(i + 1)
                    j2 = scr.tile([P, 2 * qmax], dt, tag="j2", name="j2")[:, 0:2 * q]
                    nc.scalar.activation(out=j2, in_=u_sc, func=AF.Square,
                                         accum_out=s_p0[:, i:i + 1])
                    j4b = scr.tile([P, 4 * qmax], dt, tag="j4b", name="j4b")[:, 0:4 * q]
                    nc.scalar.activation(out=j4b, in_=y, func=AF.Abs,
                                         bias=bneg[:], accum_out=s_ext[:, i:i + 1])
                    j3 = scr.tile([P, 3 * qmax], dt, tag="j3", name="j3")[:, 0:3 * q]
                    nc.scalar.activation(out=j3, in_=w, func=AF.Square,
                                         accum_out=s_p2[:, i:i + 1])

            out_sb = acc.tile([P, 8], f32, tag="out_sb")
            nc.vector.memset(out_sb[:], 0.0)
            for j, s in enumerate([s_p0, s_p1, s_p2, s_ext, s_nrm]):
                nc.vector.tensor_reduce(out=out_sb[:, j:j + 1], in_=s[:],
                                        axis=mybir.AxisListType.X,
                                        op=mybir.AluOpType.add)
            nc.sync.dma_start(out=out_d[:], in_=out_sb[:])

    nc.compile()
    return nc


def _prep(arr, shard, core, q=Q, half=True, bias_angle=True):
    # [B, 9] row-major -> per-core [n_units*P, 9*128]: each tile's [P, 9*q_i]
    # block (PERM col order, angle cols -0.5) stored contiguous in dest byte
    # order, so every tile's DMA is a dense row-range memcpy.
    sl = arr[core * shard:(core + 1) * shard, :]
    qs = _qs(shard)
    U = 128
    flat = np.empty(shard * 9, dtype=np.float16 if half else np.float32)
    r = 0
    off = 0
    for q in qs:
        a = sl[r * P:(r + q) * P].reshape(P, q, 9).transpose(0, 2, 1)[:, PERM, :]
        a = np.ascontiguousarray(a, dtype=np.float32)
        if bias_angle:
            a[:, 5:9, :] -= 0.5
        flat[off:off + P * 9 * q] = a.reshape(-1).astype(flat.dtype)
        r += q
        off += P * 9 * q
    return flat.reshape(shard * 9 // (9 * U), 9 * U)


def _finish(partials, batch):
    tot = partials.astype(np.float64).sum(axis=(0, 1))
    p0, p1, p2, ext, nrm = tot[0], tot[1], tot[2], tot[3], tot[4]
    c0 = ext / batch / CONSTANT_WEIGHT
    c1 = nrm / batch / CONSTANT_WEIGHT
    mse = (p0 + p1 + p2) / (batch * 9)
    if (p0 > p1) and (p0 > p2):
        amount = 0.0
    elif (p0 > p1) and (p0 < p2):
        amount = c1
    elif (p0 < p1) and (p0 > p2):
        amount = c0
    else:
        amount = c0 + c1
    return np.float32(mse + amount)


def _run(outputs, targets, shard, q, n_tiles, n_cores, half=HALF, **spmd_kwargs):
    key = (shard, q, n_tiles, half)
    if key not in _cache:
        _cache[key] = _build(shard, q, n_tiles, half=half)
    nc = _cache[key]
    in_maps = [{"o": _prep(outputs, shard, k, q, half),
                "t": _prep(targets, shard, k, q, half)}
               for k in range(n_cores)]
    br = run_bass_kernel_spmd(nc, in_maps, list(range(n_cores)), **spmd_kwargs)
    partials = np.stack([r["partials"] for r in br.results])
    if spmd_kwargs:
        return partials, br
    return partials


def kernel(outputs, targets):
    outputs = np.asarray(outputs)
    targets = np.asarray(targets)
    assert outputs.shape == (BATCH, 9), outputs.shape
    partials = _run(outputs, targets, SHARD, Q, N_TILES, N_CORES)
    return _finish(partials, BATCH)
